# revision 1
# baseline (speedup 1.0000x reference)
"""Trainium2 Bass kernel for a 4x2048x768 no-scale no-mask attention block.

Sharding: 8 cores = 4 batches x 2 query-halves. Each core computes the
projections for its batch (K/V over the full 2048-key sequence), attention for
its 1024 queries, and the output projection. The program is SPMD-identical
across cores: the host rolls each core's copy of x along the sequence axis so
that the core's own queries always occupy columns 0:1024 — softmax attention
is invariant to a permutation of the keys, so rolling K/V is harmless.

Weight preprocessing on the host (exact algebra, weights only):
  scores  S[i,j] = (x_i Wq^T + bq)(x_j Wk^T + bk)^T
                 = x_i A x_j^T + w[j] + u[i] + c      with A = Wq^T Wk,
                   w = x (Wk^T bq),  u = x (Wq^T bk),  c = bq.bk
  u[i] and c are constant along the softmax axis j, so they cancel after
  normalization and are dropped. The kernel computes H = x A^T (one
  k-style projection) and S^T tiles [keys, queries] = HT x xT directly —
  the whole q-projection disappears. w is a tiny device-side matvec applied
  as the per-partition bias of the exp activation.
  bv folds exactly into bo_eff = bo + wo @ bv because softmax rows sum to 1.

Device-side layout: all matmul contractions keep the contracted dim on SBUF
partitions. exp(S^T) tiles feed the P@V matmul as stationary operands giving
yT [h, i]; row-sums of exp(S^T) come from a ones-column matmul riding the
same moving operand (M=8: M=1 matmuls are ~1.7x slower). Softmax
normalization is deferred to the very end:
out = (y_unnorm @ woT + Z x bo_eff) * (1/Z), with the bias applied as a
Z-scaled rank-1 matmul into the same PSUM accumulation. Big matmuls run as
float32r (full PE rate, 4x fp32); accumulation is fp32 in PSUM.
"""

import sys

if "/opt/trn_rl_repo" not in sys.path:
    sys.path.insert(0, "/opt/trn_rl_repo")

import numpy as np

B = 4
S = 2048
D = 768
DT = D // 128  # 6 feature tiles
QH = 1024  # queries per core
NCORES = 8

_CACHE = {}
last_results = None  # BassKernelResults of the most recent run (for test harness)


def _build_nc():
    if "nc" in _CACHE:
        return _CACHE["nc"]

    from concourse import bacc, mybir
    import concourse.tile as tile

    f32 = mybir.dt.float32
    f32r = mybir.dt.float32r
    AF = mybir.ActivationFunctionType

    nc = bacc.Bacc("TRN2", target_bir_lowering=False, debug=False)

    def dram(name, shape, kind, dt=f32):
        return nc.dram_tensor(name, list(shape), dt, kind=kind).ap()

    xT = dram("xT", (DT, 128, S), "ExternalInput", f32r)  # x[b].T rolled, d-tiled
    waT = dram("waT", (DT, 128, D), "ExternalInput", f32r)  # (Wq^T Wk)^T tiles
    wvT = dram("wvT", (DT, 128, D), "ExternalInput", f32r)
    woT = dram("woT", (DT, 128, D), "ExternalInput", f32r)
    wkbq = dram("wkbq", (DT, 128, 8), "ExternalInput", f32r)  # Wk^T bq, x8 cols
    boe = dram("boe", (1, D), "ExternalInput", f32r)  # bo + wo @ bv
    out = dram("out", (QH, D), "ExternalOutput")

    NJC = S // 512  # 4 column sweeps
    NJ = S // 128  # 16 key tiles

    with tile.TileContext(nc) as tc:
        # ---- long-lived constants (left side) ----
        consts = tc.alloc_tile_pool(name="consts", bufs=1, side="left")
        ones_f = consts.tile([128, 8], f32, tag="ones_f", name="ones_f")
        nc.vector.memset(ones_f, 1.0)
        ones = consts.tile([128, 8], f32r, tag="ones", name="ones")
        nc.vector.tensor_copy(ones, ones_f)
        boe_sb = consts.tile([1, D], f32r, tag="boe", name="boe_sb")
        nc.sync.dma_start(out=boe_sb, in_=boe)
        onesr_f = consts.tile([1, 128], f32, tag="onesr_f", name="onesr_f")
        nc.vector.memset(onesr_f, 1.0)
        onesr = consts.tile([1, 128], f32r, tag="onesr", name="onesr")
        nc.vector.tensor_copy(onesr, onesr_f)
        boe_bc = consts.tile([128, D], f32, tag="boe_bc", name="boe_bc")
        wkbq_sb = consts.tile([128, DT * 8], f32r, tag="wkbq", name="wkbq_sb")
        for d in range(DT):
            nc.sync.dma_start(out=wkbq_sb[:, d * 8 : (d + 1) * 8], in_=wkbq[d])

        # ---- phase inputs (right side) ----
        # x stays resident through attention (the S^T matmuls read it).
        xpool = tc.alloc_tile_pool(name="xpool", bufs=1, side="right")
        wpool = tc.alloc_tile_pool(name="wpool", bufs=11, side="right")

        xt = [
            xpool.tile([128, S], f32r, tag=f"xt{d}", name=f"xt{d}") for d in range(DT)
        ]

        def wload(src, d, pfx, rows=2):
            # Split each weight-tile load across DMA queues by PARTITION rows:
            # pieces keep the full 3KB-per-partition contiguous lines (DMA
            # efficiency needs >=2KB lines) while one dma_start otherwise
            # lands on a single ~24GB/s queue.
            t = wpool.tile([128, D], f32r, tag="w", name=f"{pfx}{d}")
            rh = 128 // rows
            for r in range(rows):
                nc.sync.dma_start(
                    out=t[r * rh : (r + 1) * rh, :],
                    in_=src[d][r * rh : (r + 1) * rh, :],
                )
            return t

        def xchunk(d, jc, rows=1):
            lo = jc * 512
            rh = 128 // rows
            for r in range(rows):
                nc.sync.dma_start(
                    out=xt[d][r * rh : (r + 1) * rh, lo : lo + 512],
                    in_=xT[d][r * rh : (r + 1) * rh, lo : lo + 512],
                )

        wa = []
        for d in range(DT):  # what the first HT group reads, in read order
            wa.append(wload(waT, d, "wa", rows=4 if d < 2 else 2))
            xchunk(d, 0, rows=2)
        for jc in range(1, NJC):
            for d in range(DT):
                xchunk(d, jc, rows=2)
        # wv is not consumed until the v-projection (~50us in): emit its DMAs
        # after all x chunks so it doesn't contend with the HT-critical bytes.
        wv = [wload(wvT, d, "wv") for d in range(DT)]

        # ---- P2: HT[h, j] = (x A^T)^T over the full (rolled) sequence,
        #      plus the w-row matvec riding the same x chunks ----
        hpool = tc.alloc_tile_pool(name="hpool", bufs=1, side="left")
        ht = [
            hpool.tile([128, S], f32r, tag=f"ht{h}", name=f"ht{h}") for h in range(DT)
        ]
        wbias = tc.alloc_tile_pool(name="wbias", bufs=1, side="left")
        wcol = wbias.tile([128, NJ], f32, tag="wcol", name="wcol")

        stps = tc.alloc_tile_pool(name="stps", bufs=1, space="PSUM")
        expool = tc.alloc_tile_pool(name="expool", bufs=4, side="left")
        paA = tc.alloc_tile_pool(name="paA", bufs=7, space="PSUM")
        for jc in range(NJC):
            hps = [
                paA.tile([128, 512], f32, tag="pa", name=f"hps{jc}_{h}")
                for h in range(DT)
            ]
            wps = paA.tile([8, 512], f32, tag="pa", name=f"wps{jc}")
            for d in range(DT):
                for h in range(DT):
                    nc.tensor.matmul(
                        hps[h],
                        wa[d][:, h * 128 : (h + 1) * 128],
                        xt[d][:, jc * 512 : (jc + 1) * 512],
                        start=(d == 0),
                        stop=(d == DT - 1),
                    )
                nc.tensor.matmul(
                    wps,
                    wkbq_sb[:, d * 8 : (d + 1) * 8],
                    xt[d][:, jc * 512 : (jc + 1) * 512],
                    start=(d == 0),
                    stop=(d == DT - 1),
                )
            for h in range(DT):
                nc.scalar.activation(
                    ht[h][:, jc * 512 : (jc + 1) * 512], hps[h], AF.Copy
                )
            wr = wbias.tile([1, 512], f32, tag="wrow", name=f"wr{jc}", bufs=2)
            nc.vector.tensor_copy(wr, wps[0:1, :])
            for t in range(4):
                nc.sync.dma_start(
                    out=wcol[:, jc * 4 + t : jc * 4 + t + 1],
                    in_=wr[0:1, t * 128 : (t + 1) * 128],
                )
        paA.release()

        zps_pool = tc.alloc_tile_pool(name="zps", bufs=1, space="PSUM")

        # Broadcast bo_eff across all 128 partitions once: rank-1 matmul
        # ones-column x boe row, copied to SBUF. The out-projection then adds
        # it on the vector engine instead of 16 rank-1 PE matmuls.
        for lo, w in ((0, 512), (512, 256)):
            bbp = stps.tile([128, w], f32, tag="st", name=f"bbp{lo}")
            nc.tensor.matmul(
                bbp, onesr, boe_sb[0:1, lo : lo + w], start=True, stop=True
            )
            nc.vector.tensor_copy(boe_bc[:, lo : lo + w], bbp)

        # Pre-emit the first few S^T tiles + exp of attention block 0: their
        # inputs (ht, xt) are ready, so they fill the PE during the phase
        # boundary and the v-projection's DVE drains.
        pre_ex = []
        for j in range(3):
            stp = stps.tile([128, 512], f32, tag="st", name=f"st0_{j}")
            for d in range(DT):
                nc.tensor.matmul(
                    stp,
                    ht[d][:, j * 128 : (j + 1) * 128],
                    xt[d][:, 0:512],
                    start=(d == 0),
                    stop=(d == DT - 1),
                )
            ex = expool.tile([128, 512], f32r, tag="ex", name=f"ex0_{j}")
            nc.scalar.activation(ex, stp, AF.Exp, bias=wcol[:, j : j + 1])
            pre_ex.append(ex)

        # ---- P4: v[s, h] token-major, packed as [128, 16*768] ----
        vpool = tc.alloc_tile_pool(name="vpool", bufs=1, side="left")
        v_all = vpool.tile([128, NJ * D], f32r, tag="v", name="v_all")
        paB = tc.alloc_tile_pool(name="paB", bufs=2, space="PSUM")
        for s in range(NJ):
            vps = paB.tile([128, D], f32, tag="pb", name=f"vps{s}")
            for d in range(DT):
                nc.tensor.matmul(
                    vps[:, 0:512],
                    xt[d][:, s * 128 : (s + 1) * 128],
                    wv[d][:, 0:512],
                    start=(d == 0),
                    stop=(d == DT - 1),
                )
                nc.tensor.matmul(
                    vps[:, 512:768],
                    xt[d][:, s * 128 : (s + 1) * 128],
                    wv[d][:, 512:768],
                    start=(d == 0),
                    stop=(d == DT - 1),
                )
            nc.vector.tensor_copy(v_all[:, s * D : (s + 1) * D], vps)
        paB.release()
        wpool.release()

        # ---- out-projection weights (left, loads overlap attention) ----
        wopool = tc.alloc_tile_pool(name="wopool", bufs=1, side="left")
        wo = []
        for h in range(DT):
            t = wopool.tile([128, D], f32r, tag=f"wo{h}", name=f"wo{h}")
            for r in range(2):
                nc.sync.dma_start(
                    out=t[r * 64 : (r + 1) * 64, :], in_=woT[h][r * 64 : (r + 1) * 64, :]
                )
            wo.append(t)

        # ---- P5: attention + out-projection, per 512-query block ----
        ytpool = tc.alloc_tile_pool(name="ytpool", bufs=1, side="left")
        zpool = tc.alloc_tile_pool(name="zpool", bufs=2, side="left")
        outpool = tc.alloc_tile_pool(name="outpool", bufs=2, side="left")
        pyps = tc.alloc_tile_pool(name="pyps", bufs=6, space="PSUM")

        def emit_st(ib, j):
            io = ib * 512
            stp = stps.tile([128, 512], f32, tag="st", name=f"st{ib}_{j}")
            for d in range(DT):
                nc.tensor.matmul(
                    stp,
                    ht[d][:, j * 128 : (j + 1) * 128],
                    xt[d][:, io : io + 512],
                    start=(d == 0),
                    stop=(d == DT - 1),
                )
            ex = expool.tile([128, 512], f32r, tag="ex", name=f"ex{ib}_{j}")
            nc.scalar.activation(ex, stp, AF.Exp, bias=wcol[:, j : j + 1])
            return ex

        nxt_ex = pre_ex
        for ib in range(QH // 512):
            io = ib * 512
            yps = [
                pyps.tile([128, 512], f32, tag="py", name=f"yps{ib}_{h}")
                for h in range(DT)
            ]
            zp = zps_pool.tile([8, 512], f32, tag="pz", name=f"zp{ib}")

            # Software-pipelined: the PV/rowsum matmuls lag the S^T matmuls
            # by `lag` steps, so the PE chews on them while the ACT exp runs.
            exq = list(nxt_ex)
            lag = 3
            j0 = 3

            def consume(jd, zp=zp, yps=yps):
                exd = exq.pop(0)
                nc.tensor.matmul(zp, ones, exd, start=(jd == 0), stop=(jd == NJ - 1))
                for h in range(DT):
                    nc.tensor.matmul(
                        yps[h],
                        v_all[:, jd * D + h * 128 : jd * D + (h + 1) * 128],
                        exd,
                        start=(jd == 0),
                        stop=(jd == NJ - 1),
                    )

            for j in range(j0, NJ):
                exq.append(emit_st(ib, j))
                if j >= lag:
                    consume(j - lag)
            for jd in range(NJ - lag, NJ):
                consume(jd)
            # Pre-emit the next block's first S^T tiles so the PE has work
            # during this block's Z/yT drains and out-projection waits.
            nxt_ex = []
            if ib == 0:
                nxt_ex = [emit_st(ib + 1, j) for j in range(3)]

            # Z row -> SBUF (f32r copy for the rank-1 bias matmul, f32 copy
            # for the transpose); scatter-transpose the row to per-partition
            # columns with SBUF->SBUF DMAs; reciprocal for the final scale.
            z_f = zpool.tile([1, 512], f32, tag="zf", name=f"z_f{ib}")
            nc.vector.tensor_copy(z_f, zp[0:1, :])
            zcol = zpool.tile([128, 4], f32, tag="zc", name=f"zcol{ib}")
            for t in range(4):
                nc.sync.dma_start(
                    out=zcol[:, t : t + 1], in_=z_f[0:1, t * 128 : (t + 1) * 128]
                )
            rz = zpool.tile([128, 4], f32, tag="rz", name=f"rz{ib}")
            nc.vector.reciprocal(rz, zcol)

            yt = ytpool.tile([128, DT * 512], f32r, tag="yt", name=f"yt{ib}")
            for h in range(DT):
                nc.vector.tensor_copy(yt[:, h * 512 : (h + 1) * 512], yps[h])

            for t in range(4):
                opa = pyps.tile([128, 512], f32, tag="py", name=f"opa{ib}_{t}")
                opb = pyps.tile([128, 256], f32, tag="py", name=f"opb{ib}_{t}")
                for h in range(DT):
                    lhs = yt[:, h * 512 + t * 128 : h * 512 + (t + 1) * 128]
                    nc.tensor.matmul(
                        opa, lhs, wo[h][:, 0:512], start=(h == 0), stop=(h == DT - 1)
                    )
                    nc.tensor.matmul(
                        opb, lhs, wo[h][:, 512:768], start=(h == 0), stop=(h == DT - 1)
                    )
                osb = outpool.tile([128, D], f32, tag="ot", name=f"osb{ib}_{t}")
                ro = io + t * 128
                for p in range(3):
                    sl = slice(p * 256, (p + 1) * 256)
                    ps = opa[:, sl] if p < 2 else opb
                    nc.vector.tensor_scalar_mul(osb[:, sl], ps, rz[:, t : t + 1])
                    nc.vector.tensor_add(osb[:, sl], osb[:, sl], boe_bc[:, sl])
                    nc.sync.dma_start(out=out[ro : ro + 128, sl], in_=osb[:, sl])

        for p in (pyps, outpool, zpool, ytpool, wopool, vpool, expool,
                  zps_pool, stps, wbias, hpool, xpool, consts):
            p.release()

    nc.compile()
    _CACHE["nc"] = nc
    return nc


def _shard_inputs(x, wq, bq, wk, bk, wv, bv, wo, bo):
    """Build the 8 per-core input maps (host-side layout + weight algebra)."""
    f = np.float32
    f8 = np.float64
    x = np.asarray(x, f)
    wq, wk, wv, wo = (np.asarray(a, f) for a in (wq, wk, wv, wo))
    bq, bk, bv, bo = (np.asarray(a, f) for a in (bq, bk, bv, bo))

    def wtiles(w):  # torch Linear weight [out, in] -> [in-tile, 128, out]
        return np.ascontiguousarray(np.asarray(w, f).T).reshape(DT, 128, D)

    A = (wq.astype(f8).T @ wk.astype(f8)).astype(f)  # [d, e]; H = x @ A.T
    wkbq_col = (wk.astype(f8).T @ bq.astype(f8)).astype(f)  # [768]
    shared = {
        "waT": wtiles(A),
        "wvT": wtiles(wv),
        "woT": wtiles(wo),
        "wkbq": np.ascontiguousarray(
            np.repeat(wkbq_col.reshape(DT, 128, 1), 8, axis=2)
        ),
        "boe": (bo.astype(f8) + wo.astype(f8) @ bv.astype(f8)).astype(f).reshape(1, D),
    }
    in_maps = []
    for c in range(NCORES):
        b, half = c // 2, c % 2
        xb = np.ascontiguousarray(x[b].T)  # [D, S]
        if half:
            xb = np.concatenate([xb[:, QH:], xb[:, :QH]], axis=1)
        m = dict(shared)
        m["xT"] = np.ascontiguousarray(xb).reshape(DT, 128, S)
        in_maps.append(m)
    return in_maps


def kernel(x, wq, bq, wk, bk, wv, bv, wo, bo, trace=False, trace_kwargs=None):
    global last_results
    from concourse.bass_utils import run_bass_kernel_spmd

    nc = _build_nc()
    in_maps = _shard_inputs(x, wq, bq, wk, bk, wv, bv, wo, bo)
    res = run_bass_kernel_spmd(
        nc,
        in_maps,
        core_ids=list(range(NCORES)),
        trace=trace,
        **(trace_kwargs or {}),
    )
    last_results = res
    out = np.empty((B, S, D), np.float32)
    for c in range(NCORES):
        b, half = c // 2, c % 2
        out[b, half * QH : (half + 1) * QH, :] = res.results[c]["out"]
    return out



# revision 9
# speedup vs baseline: 1.2260x; 1.2260x over previous
"""Trainium2 Bass kernel for a 4x2048x768 no-scale no-mask attention block.

Sharding: 8 cores = 4 batches x 2 query-halves. Each core computes the
projections for its batch (K/V over the full 2048-key sequence), attention for
its 1024 queries, and the output projection. The program is SPMD-identical
across cores: the host rolls each core's copy of x along the sequence axis so
that the core's own queries always occupy columns 0:1024 — softmax attention
is invariant to a permutation of the keys, so rolling K/V is harmless.

Weight preprocessing on the host (exact algebra, weights only):
  scores  S[i,j] = (x_i Wq^T + bq)(x_j Wk^T + bk)^T
                 = x_i A x_j^T + w[j] + u[i] + c      with A = Wq^T Wk,
                   w = x (Wk^T bq),  u = x (Wq^T bk),  c = bq.bk
  u[i] and c are constant along the softmax axis j and cancel after
  normalization. The kernel computes H = x A^T (one k-style projection) and
  S^T tiles [keys, queries] = HT x xT directly — the q-projection disappears.
  The value path and the out-projection fuse into ONE projection:
  out = P/Z @ (x Wv^T) Wo^T + (bo + Wo bv) = (1/Z) * P @ (x Wvo^T) + boe
  with Wvo = Wo Wv (softmax rows sum to 1 folds bv into boe). w rides the
  vo-projection as one extra moving column, landing token-major as the
  per-partition bias of the exp activation.

Mixed precision (validated ~4e-3 rel err vs 2e-2 budget): the score path
(x, A, ht) is fp16 — 3 extra mantissa bits over bf16 keep softmax logit noise
small; the value path (wvo, vo) is fp16 and exp(S) is bf16 (needs fp32-range
exponents). 16-bit stationaries enable Fast Weight Load so LDWEIGHTS hides
under the matmul streams (fp32 stationaries disable FWL and pace the PE).
fp16 x/weights/output also halve HBM traffic, shrinking the DMA-bound head.

Attention uses exp(S^T) tiles as the STATIONARY operand (4 query-slices of
128), each reused across three moving operands: vo columns 0:512, vo columns
512:768, and a ones column that accumulates Z. Output lands query-major
[q, d] so the 1/Z softmax scale is a per-partition tensor_scalar and the
result DMAs straight to the row-major output. PSUM: 3x [128,1024] out accums
+ S^T staging + Z = exactly 8 banks.
"""

import sys

if "/opt/trn_rl_repo" not in sys.path:
    sys.path.insert(0, "/opt/trn_rl_repo")

import numpy as np

B = 4
S = 2048
D = 768
DT = D // 128  # 6 feature tiles
QH = 1024  # queries per core
NCORES = 8
NJ = S // 128  # 16 key tiles

_CACHE = {}
last_results = None  # BassKernelResults of the most recent run (for test harness)


def _build_nc():
    if "nc" in _CACHE:
        return _CACHE["nc"]

    from concourse import bacc, mybir
    import concourse.tile as tile

    f32 = mybir.dt.float32
    f32r = mybir.dt.float32r
    f16 = mybir.dt.float16
    bf16 = mybir.dt.bfloat16
    AF = mybir.ActivationFunctionType

    nc = bacc.Bacc("TRN2", target_bir_lowering=False, debug=False)

    def dram(name, shape, kind, dt=f32):
        return nc.dram_tensor(name, list(shape), dt, kind=kind).ap()

    xT = dram("xT", (DT, 128, S), "ExternalInput", f16)  # x[b].T rolled, d-tiled
    waT = dram("waT", (DT, 128, D), "ExternalInput", f16)  # (x A^T)-style tiles
    wvoT = dram("wvoT", (DT, 128, D + 1), "ExternalInput", f16)  # [WvoT | Wk^T bq]
    boe = dram("boe", (1, D), "ExternalInput", f16)  # bo + wo @ bv
    out = dram("out", (QH, D), "ExternalOutput", f16)

    with tile.TileContext(nc) as tc:
        # ---- long-lived constants and small state (left side) ----
        consts = tc.alloc_tile_pool(name="consts", bufs=1, side="left")
        ones_f = consts.tile([128, 1], f32, tag="ones_f", name="ones_f")
        nc.vector.memset(ones_f, 1.0)
        onesc = consts.tile([128, 1], f16, tag="onesc", name="onesc")
        nc.vector.tensor_copy(onesc, ones_f)
        onesr_f = consts.tile([1, 128], f32, tag="onesr_f", name="onesr_f")
        nc.vector.memset(onesr_f, 1.0)
        onesr = consts.tile([1, 128], f16, tag="onesr", name="onesr")
        nc.vector.tensor_copy(onesr, onesr_f)
        boe_sb = consts.tile([1, D], f16, tag="boe", name="boe_sb")
        nc.sync.dma_start(out=boe_sb, in_=boe)
        boe_bc = consts.tile([128, D], f16, tag="boe_bc", name="boe_bc")
        wcol = consts.tile([128, NJ], f32, tag="wcol", name="wcol")

        # ---- phase inputs (right side) ----
        xpool = tc.alloc_tile_pool(name="xpool", bufs=1, side="right")
        wapool = tc.alloc_tile_pool(name="wapool", bufs=1, side="right")
        wvopool = tc.alloc_tile_pool(name="wvopool", bufs=1, side="right")

        xt = [
            xpool.tile([128, S], f16, tag=f"xt{d}", name=f"xt{d}") for d in range(DT)
        ]
        wa = [
            wapool.tile([128, D], f16, tag=f"wa{d}", name=f"wa{d}") for d in range(DT)
        ]
        wvo = [
            wvopool.tile([128, D + 1], f16, tag=f"wvo{d}", name=f"wvo{d}")
            for d in range(DT)
        ]

        # DMA order = consumption order: per-d weight tile + first x chunk
        # (the first HT sweep reads all six d in sequence), then the
        # remaining x chunks, then the vo weights (needed ~30us in).
        for d in range(DT):
            nc.sync.dma_start(out=wa[d], in_=waT[d])
            nc.sync.dma_start(out=xt[d][:, 0:512], in_=xT[d][:, 0:512])
        for jc in range(1, 4):
            lo = jc * 512
            for d in range(DT):
                nc.sync.dma_start(
                    out=xt[d][:, lo : lo + 512], in_=xT[d][:, lo : lo + 512]
                )
        for d in range(DT):
            nc.sync.dma_start(out=wvo[d], in_=wvoT[d])

        # ---- P1: HT[h, j] = (x A^T)^T over the full (rolled) sequence ----
        hpool = tc.alloc_tile_pool(name="hpool", bufs=1, side="left")
        ht = [
            hpool.tile([128, S], f16, tag=f"ht{h}", name=f"ht{h}") for h in range(DT)
        ]
        paA = tc.alloc_tile_pool(name="paA", bufs=7, space="PSUM")
        for jc in range(4):
            hps = [
                paA.tile([128, 512], f32, tag="pa", name=f"hps{jc}_{h}")
                for h in range(DT)
            ]
            for d in range(DT):
                for h in range(DT):
                    nc.tensor.matmul(
                        hps[h],
                        wa[d][:, h * 128 : (h + 1) * 128],
                        xt[d][:, jc * 512 : (jc + 1) * 512],
                        start=(d == 0),
                        stop=(d == DT - 1),
                    )
            for h in range(DT):
                nc.scalar.activation(
                    ht[h][:, jc * 512 : (jc + 1) * 512], hps[h], AF.Copy
                )
        paA.release()

        # ---- P2: vo[s, h] token-major fp16, plus the w bias column; also
        #      broadcast boe across partitions with a rank-1 matmul ----
        paB = tc.alloc_tile_pool(name="paB", bufs=2, space="PSUM")
        bbp = paB.tile([128, D], f32, tag="bb", name="bbp", bufs=1)
        nc.tensor.matmul(bbp[:, 0:512], onesr, boe_sb[0:1, 0:512], start=True, stop=True)
        nc.tensor.matmul(bbp[:, 512:768], onesr, boe_sb[0:1, 512:768], start=True, stop=True)
        nc.vector.tensor_copy(boe_bc, bbp)

        vpool = tc.alloc_tile_pool(name="vpool", bufs=1, side="left")
        v_all = vpool.tile([128, NJ * D], f16, tag="v", name="v_all")
        for s in range(NJ):
            vps = paB.tile([128, D + 1], f32, tag="pb", name=f"vps{s}")
            for d in range(DT):
                nc.tensor.matmul(
                    vps[:, 0:512],
                    xt[d][:, s * 128 : (s + 1) * 128],
                    wvo[d][:, 0:512],
                    start=(d == 0),
                    stop=(d == DT - 1),
                )
                nc.tensor.matmul(
                    vps[:, 512 : D + 1],
                    xt[d][:, s * 128 : (s + 1) * 128],
                    wvo[d][:, 512 : D + 1],
                    start=(d == 0),
                    stop=(d == DT - 1),
                )
            nc.vector.tensor_copy(v_all[:, s * D : (s + 1) * D], vps[:, 0:D])
            nc.vector.tensor_copy(wcol[:, s : s + 1], vps[:, D : D + 1])
        paB.release()

        # ---- P3: attention, exp(S^T) stationary, fused vo/out projection ----
        stps = tc.alloc_tile_pool(name="stps", bufs=1, space="PSUM")
        expool = tc.alloc_tile_pool(name="expool", bufs=6, side="left")
        pvps = tc.alloc_tile_pool(name="pvps", bufs=3, space="PSUM")
        zps = tc.alloc_tile_pool(name="zps", bufs=1, space="PSUM")
        rzpool = tc.alloc_tile_pool(name="rzpool", bufs=2, side="left")
        outpool = tc.alloc_tile_pool(name="outpool", bufs=3, side="left")

        for ib in range(QH // 512):
            io = ib * 512
            T0 = pvps.tile([128, 1024], f32, tag="pv", name=f"T0_{ib}")
            T1 = pvps.tile([128, 1024], f32, tag="pv", name=f"T1_{ib}")
            T2 = pvps.tile([128, 1024], f32, tag="pv", name=f"T2_{ib}")
            zp = zps.tile([128, 4], f32, tag="z", name=f"zp{ib}")

            exq = []

            def emit_st(j, ib=ib, io=io):
                stp = stps.tile([128, 512], f32, tag="st", name=f"st{ib}_{j}")
                for d in range(DT):
                    nc.tensor.matmul(
                        stp,
                        ht[d][:, j * 128 : (j + 1) * 128],
                        xt[d][:, io : io + 512],
                        start=(d == 0),
                        stop=(d == DT - 1),
                    )
                ex = expool.tile([128, 512], bf16, tag="ex", name=f"ex{ib}_{j}")
                nc.scalar.activation(ex, stp, AF.Exp, bias=wcol[:, j : j + 1])
                return ex

            def consume(jd, T0=T0, T1=T1, T2=T2, zp=zp, exq=exq):
                # PSUM start_tensor_calc clears the enclosing 2KB BANK, so a
                # bank hosting several column-interleaved accumulation groups
                # must be started exactly once (first group) and stopped once
                # (last group); co-bank groups land on pending-zero bytes.
                exd = exq.pop(0)
                st = (jd == 0)
                sp = (jd == NJ - 1)
                for t in range(4):
                    exsl = exd[:, t * 128 : (t + 1) * 128]
                    Tq = T0 if t < 2 else T1
                    qo = (t % 2) * 512
                    nc.tensor.matmul(
                        Tq[:, qo : qo + 512],
                        exsl,
                        v_all[:, jd * D : jd * D + 512],
                        start=st,
                        stop=sp,
                    )
                    nc.tensor.matmul(
                        T2[:, t * 256 : (t + 1) * 256],
                        exsl,
                        v_all[:, jd * D + 512 : jd * D + 768],
                        start=st and t in (0, 2),
                        stop=sp and t in (1, 3),
                        skip_group_check=True,
                    )
                    nc.tensor.matmul(
                        zp[:, t : t + 1],
                        exsl,
                        onesc,
                        start=st and t == 0,
                        stop=sp and t == 3,
                        skip_group_check=True,
                    )

            lag = 2
            for j in range(NJ):
                exq.append(emit_st(j))
                if j >= lag:
                    consume(j - lag)
            for jd in range(NJ - lag, NJ):
                consume(jd)

            rz = rzpool.tile([128, 4], f32, tag="rz", name=f"rz{ib}")
            nc.vector.reciprocal(rz, zp)
            for t in range(4):
                osb = outpool.tile([128, D], f16, tag="ot", name=f"osb{ib}_{t}")
                Tq = T0 if t < 2 else T1
                qo = (t % 2) * 512
                nc.vector.tensor_scalar_mul(
                    osb[:, 0:512], Tq[:, qo : qo + 512], rz[:, t : t + 1]
                )
                nc.vector.tensor_scalar_mul(
                    osb[:, 512:768], T2[:, t * 256 : (t + 1) * 256], rz[:, t : t + 1]
                )
                nc.vector.tensor_add(osb[:, 0:512], osb[:, 0:512], boe_bc[:, 0:512])
                nc.vector.tensor_add(
                    osb[:, 512:768], osb[:, 512:768], boe_bc[:, 512:768]
                )
                ro = io + t * 128
                nc.sync.dma_start(out=out[ro : ro + 128, :], in_=osb)

        for p in (outpool, rzpool, zps, pvps, expool, stps, vpool, hpool,
                  wvopool, wapool, xpool, consts):
            p.release()

    nc.compile()
    _CACHE["nc"] = nc
    return nc


def _shard_inputs(x, wq, bq, wk, bk, wv, bv, wo, bo):
    """Build the 8 per-core input maps (host-side layout + weight algebra)."""
    f = np.float32
    f8 = np.float64
    h = np.float16
    x = np.asarray(x, f)
    wq, wk, wv, wo = (np.asarray(a, f8) for a in (wq, wk, wv, wo))
    bq, bk, bv, bo = (np.asarray(a, f8) for a in (bq, bk, bv, bo))

    def wtiles(w, dt):  # weight [out, in] -> [in-tile, 128, out]
        return np.ascontiguousarray(np.asarray(w, f).T).reshape(DT, 128, -1).astype(dt)

    A = (wq.T @ wk).astype(f)  # [d, e]; H = x @ A.T
    wvo = (wo @ wv).astype(f)  # fused value+out projection
    wkbq_col = (wk.T @ bq).astype(f)  # [768] -> w = x @ wkbq
    wvoT = wtiles(wvo, h)  # (DT, 128, D)
    wvoT_aug = np.concatenate(
        [wvoT, wkbq_col.reshape(DT, 128, 1).astype(h)], axis=2
    )  # (DT, 128, D+1)
    shared = {
        "waT": wtiles(A, h),
        "wvoT": np.ascontiguousarray(wvoT_aug),
        "boe": (bo + wo @ bv).astype(h).reshape(1, D),
    }
    in_maps = []
    for c in range(NCORES):
        b, half = c // 2, c % 2
        xb = np.ascontiguousarray(x[b].T)  # [D, S]
        if half:
            xb = np.concatenate([xb[:, QH:], xb[:, :QH]], axis=1)
        m = dict(shared)
        m["xT"] = np.ascontiguousarray(xb).reshape(DT, 128, S).astype(h)
        in_maps.append(m)
    return in_maps


def kernel(x, wq, bq, wk, bk, wv, bv, wo, bo, trace=False, trace_kwargs=None):
    global last_results
    from concourse.bass_utils import run_bass_kernel_spmd

    nc = _build_nc()
    in_maps = _shard_inputs(x, wq, bq, wk, bk, wv, bv, wo, bo)
    res = run_bass_kernel_spmd(
        nc,
        in_maps,
        core_ids=list(range(NCORES)),
        trace=trace,
        **(trace_kwargs or {}),
    )
    last_results = res
    out = np.empty((B, S, D), np.float32)
    for c in range(NCORES):
        b, half = c // 2, c % 2
        out[b, half * QH : (half + 1) * QH, :] = res.results[c]["out"].astype(np.float32)
    return out


# revision 15
# speedup vs baseline: 1.4774x; 1.2051x over previous
"""Trainium2 Bass kernel for a 4x2048x768 no-scale no-mask attention block.

Sharding: 8 cores = 4 batches x 2 query-halves. Each core computes the
projections for its batch (K/V over the full 2048-key sequence), attention for
its 1024 queries, and the output projection. The program is SPMD-identical
across cores: the host rolls each core's copy of x along the sequence axis so
that the core's own queries always occupy columns 0:1024 — softmax attention
is invariant to a permutation of the keys, so rolling K/V is harmless.

Weight preprocessing on the host (exact algebra, weights only):
  scores  S[i,j] = (x_i Wq^T + bq)(x_j Wk^T + bk)^T
                 = x_i A x_j^T + w[j] + u[i] + c      with A = Wq^T Wk,
                   w = x (Wk^T bq),  u = x (Wq^T bk),  c = bq.bk
  u[i] and c are constant along the softmax axis j and cancel after
  normalization. The kernel computes H = x A^T (one k-style projection) and
  S^T tiles [keys, queries] = HT x xT directly — the q-projection disappears.
  The value path and the out-projection fuse into ONE projection:
  out = P/Z @ (x Wv^T) Wo^T + (bo + Wo bv) = (1/Z) * P @ (x Wvo^T) + boe
  with Wvo = Wo Wv (softmax rows sum to 1 folds bv into boe). w rides the
  vo-projection as one extra moving column, landing token-major as the
  per-partition bias of the exp activation.

Mixed precision (validated ~4e-3 rel err vs 2e-2 budget): the score path
(x, A, ht) is fp16 — 3 extra mantissa bits over bf16 keep softmax logit noise
small; the value path (wvo, vo) is fp16 and exp(S) is bf16 (needs fp32-range
exponents). 16-bit stationaries enable Fast Weight Load so LDWEIGHTS hides
under the matmul streams (fp32 stationaries disable FWL and pace the PE).
fp16 x/weights/output also halve HBM traffic, shrinking the DMA-bound head.

Attention uses exp(S^T) tiles as the STATIONARY operand (4 query-slices of
128), each reused across three moving operands: vo columns 0:512, vo columns
512:768, and a ones column that accumulates Z. Output lands query-major
[q, d] so the 1/Z softmax scale is a per-partition tensor_scalar and the
result DMAs straight to the row-major output. PSUM: 3x [128,1024] out accums
+ S^T staging + Z = exactly 8 banks.
"""

import sys

if "/opt/trn_rl_repo" not in sys.path:
    sys.path.insert(0, "/opt/trn_rl_repo")

import numpy as np

B = 4
S = 2048
D = 768
DT = D // 128  # 6 feature tiles
QH = 1024  # queries per core
NCORES = 8
NJ = S // 128  # 16 key tiles

_CACHE = {}
last_results = None  # BassKernelResults of the most recent run (for test harness)


def _build_nc():
    if "nc" in _CACHE:
        return _CACHE["nc"]

    from concourse import bacc, mybir
    import concourse.tile as tile

    f32 = mybir.dt.float32
    f32r = mybir.dt.float32r
    f16 = mybir.dt.float16
    bf16 = mybir.dt.bfloat16
    AF = mybir.ActivationFunctionType

    nc = bacc.Bacc("TRN2", target_bir_lowering=False, debug=False)

    def dram(name, shape, kind, dt=f32):
        return nc.dram_tensor(name, list(shape), dt, kind=kind).ap()

    xT = dram("xT", (DT, 128, S), "ExternalInput", f16)  # x[b].T rolled, d-tiled
    waT = dram("waT", (DT, 128, D), "ExternalInput", f16)  # (x A^T)-style tiles
    wvoT = dram("wvoT", (DT, 128, D + 1), "ExternalInput", f16)  # [WvoT | Wk^T bq]
    boe = dram("boe", (1, D), "ExternalInput", f16)  # bo + wo @ bv
    out = dram("out", (QH, D), "ExternalOutput", f16)

    with tile.TileContext(nc) as tc:
        # ---- long-lived constants and small state (left side) ----
        consts = tc.alloc_tile_pool(name="consts", bufs=1, side="left")
        ones_f = consts.tile([128, 1], f32, tag="ones_f", name="ones_f")
        nc.vector.memset(ones_f, 1.0)
        onesc = consts.tile([128, 1], f16, tag="onesc", name="onesc")
        nc.vector.tensor_copy(onesc, ones_f)
        onesr_f = consts.tile([1, 128], f32, tag="onesr_f", name="onesr_f")
        nc.vector.memset(onesr_f, 1.0)
        onesr = consts.tile([1, 128], f16, tag="onesr", name="onesr")
        nc.vector.tensor_copy(onesr, onesr_f)
        boe_sb = consts.tile([1, D], f16, tag="boe", name="boe_sb")
        boe_bc = consts.tile([128, D], f16, tag="boe_bc", name="boe_bc")
        wcol = consts.tile([128, NJ], f32, tag="wcol", name="wcol")

        # ---- phase inputs (right side) ----
        xpool = tc.alloc_tile_pool(name="xpool", bufs=1, side="right")
        wapool = tc.alloc_tile_pool(name="wapool", bufs=1, side="right")
        wvopool = tc.alloc_tile_pool(name="wvopool", bufs=1, side="right")

        xt = [
            xpool.tile([128, S], f16, tag=f"xt{d}", name=f"xt{d}") for d in range(DT)
        ]
        wa = [
            wapool.tile([128, D], f16, tag=f"wa{d}", name=f"wa{d}") for d in range(DT)
        ]
        wvo = [
            wvopool.tile([128, D + 1], f16, tag=f"wvo{d}", name=f"wvo{d}")
            for d in range(DT)
        ]

        # DMA issue is serialized per DGE queue (~0.7us per dma_start), so
        # spread the input streams over three queues in consumption order:
        # sync carries the weights (+ later the outputs), scalar the first
        # x chunks the HT sweep needs immediately, gpsimd (SWDGE) the rest.
        for d in range(DT):
            nc.sync.dma_start(out=wa[d], in_=waT[d])
            nc.scalar.dma_start(out=xt[d][:, 0:512], in_=xT[d][:, 0:512])
        nc.sync.dma_start(out=boe_sb, in_=boe)
        for d in range(DT):
            nc.sync.dma_start(out=wvo[d], in_=wvoT[d])
        for jc in range(1, 4):
            lo = jc * 512
            for d in range(DT):
                nc.gpsimd.dma_start(
                    out=xt[d][:, lo : lo + 512], in_=xT[d][:, lo : lo + 512]
                )

        # ---- P1: HT[h, j] = (x A^T)^T over the full (rolled) sequence ----
        hpool = tc.alloc_tile_pool(name="hpool", bufs=1, side="left")
        ht = [
            hpool.tile([128, S], f16, tag=f"ht{h}", name=f"ht{h}") for h in range(DT)
        ]
        # stps lives for the whole kernel so the first S^T matmuls don't
        # inherit a write-after-read hazard from the released paB bank.
        stps = tc.alloc_tile_pool(name="stps", bufs=1, space="PSUM")
        paA = tc.alloc_tile_pool(name="paA", bufs=7, space="PSUM")
        for jc in range(4):
            hps = [
                paA.tile([128, 512], f32, tag="pa", name=f"hps{jc}_{h}")
                for h in range(DT)
            ]
            for d in range(DT):
                for h in range(DT):
                    nc.tensor.matmul(
                        hps[h],
                        wa[d][:, h * 128 : (h + 1) * 128],
                        xt[d][:, jc * 512 : (jc + 1) * 512],
                        start=(d == 0),
                        stop=(d == DT - 1),
                    )
            for h in range(DT):
                nc.scalar.activation(
                    ht[h][:, jc * 512 : (jc + 1) * 512], hps[h], AF.Copy
                )
        paA.release()

        # ---- P2: vo[s, h] token-major fp16, plus the w bias column; also
        #      broadcast boe across partitions with a rank-1 matmul ----
        paB = tc.alloc_tile_pool(name="paB", bufs=2, space="PSUM")
        bbp = paB.tile([128, D], f32, tag="bb", name="bbp", bufs=1)
        nc.tensor.matmul(bbp[:, 0:512], onesr, boe_sb[0:1, 0:512], start=True, stop=True)
        nc.tensor.matmul(bbp[:, 512:768], onesr, boe_sb[0:1, 512:768], start=True, stop=True)
        nc.vector.tensor_copy(boe_bc, bbp)

        vpool = tc.alloc_tile_pool(name="vpool", bufs=1, side="left")
        v_all = vpool.tile([128, NJ * D], f16, tag="v", name="v_all")
        for s in range(NJ):
            vps = paB.tile([128, D + 1], f32, tag="pb", name=f"vps{s}")
            for d in range(DT):
                nc.tensor.matmul(
                    vps[:, 0:512],
                    xt[d][:, s * 128 : (s + 1) * 128],
                    wvo[d][:, 0:512],
                    start=(d == 0),
                    stop=(d == DT - 1),
                )
                nc.tensor.matmul(
                    vps[:, 512 : D + 1],
                    xt[d][:, s * 128 : (s + 1) * 128],
                    wvo[d][:, 512 : D + 1],
                    start=(d == 0),
                    stop=(d == DT - 1),
                )
            nc.vector.tensor_copy(v_all[:, s * D : (s + 1) * D], vps[:, 0:D])
            nc.vector.tensor_copy(wcol[:, s : s + 1], vps[:, D : D + 1])
        paB.release()

        # ---- P3: attention, exp(S^T) stationary, fused vo/out projection ----
        expool = tc.alloc_tile_pool(name="expool", bufs=6, side="left")
        pvps = tc.alloc_tile_pool(name="pvps", bufs=3, space="PSUM")
        zps = tc.alloc_tile_pool(name="zps", bufs=1, space="PSUM")
        rzpool = tc.alloc_tile_pool(name="rzpool", bufs=2, side="left")
        outpool = tc.alloc_tile_pool(name="outpool", bufs=3, side="left")

        for ib in range(QH // 512):
            io = ib * 512
            T0 = pvps.tile([128, 1024], f32, tag="pv", name=f"T0_{ib}")
            T1 = pvps.tile([128, 1024], f32, tag="pv", name=f"T1_{ib}")
            T2 = pvps.tile([128, 1024], f32, tag="pv", name=f"T2_{ib}")
            zp = zps.tile([128, 4], f32, tag="z", name=f"zp{ib}")

            exq = []

            def emit_st(j, ib=ib, io=io):
                stp = stps.tile([128, 512], f32, tag="st", name=f"st{ib}_{j}")
                for d in range(DT):
                    nc.tensor.matmul(
                        stp,
                        ht[d][:, j * 128 : (j + 1) * 128],
                        xt[d][:, io : io + 512],
                        start=(d == 0),
                        stop=(d == DT - 1),
                    )
                ex = expool.tile([128, 512], bf16, tag="ex", name=f"ex{ib}_{j}")
                nc.scalar.activation(ex, stp, AF.Exp, bias=wcol[:, j : j + 1])
                return ex

            def consume(jd, T0=T0, T1=T1, T2=T2, zp=zp, exq=exq):
                # PSUM start_tensor_calc clears the enclosing 2KB BANK, so a
                # bank hosting several column-interleaved accumulation groups
                # must be started exactly once (first group) and stopped once
                # (last group); co-bank groups land on pending-zero bytes.
                exd = exq.pop(0)
                st = (jd == 0)
                sp = (jd == NJ - 1)
                if sp:
                    # Finish Z first: the reciprocal + output drain then
                    # overlap the remaining eight PV matmuls.
                    for t in range(4):
                        nc.tensor.matmul(
                            zp[:, t : t + 1],
                            exd[:, t * 128 : (t + 1) * 128],
                            onesc,
                            start=False,
                            stop=(t == 3),
                            skip_group_check=True,
                        )
                for t in range(4):
                    exsl = exd[:, t * 128 : (t + 1) * 128]
                    Tq = T0 if t < 2 else T1
                    qo = (t % 2) * 512
                    nc.tensor.matmul(
                        Tq[:, qo : qo + 512],
                        exsl,
                        v_all[:, jd * D : jd * D + 512],
                        start=st,
                        stop=sp,
                    )
                    nc.tensor.matmul(
                        T2[:, t * 256 : (t + 1) * 256],
                        exsl,
                        v_all[:, jd * D + 512 : jd * D + 768],
                        start=st and t in (0, 2),
                        stop=sp and t in (1, 3),
                        skip_group_check=True,
                    )
                    if not sp:
                        nc.tensor.matmul(
                            zp[:, t : t + 1],
                            exsl,
                            onesc,
                            start=st and t == 0,
                            stop=False,
                            skip_group_check=True,
                        )

            lag = 2
            for j in range(NJ):
                exq.append(emit_st(j))
                if j >= lag:
                    consume(j - lag)
            for jd in range(NJ - lag, NJ):
                consume(jd)

            rz = rzpool.tile([128, 4], f32, tag="rz", name=f"rz{ib}")
            nc.vector.reciprocal(rz, zp)
            last = ib == QH // 512 - 1
            for t in range(4):
                osb = outpool.tile([128, D], f16, tag="ot", name=f"osb{ib}_{t}")
                Tq = T0 if t < 2 else T1
                qo = (t % 2) * 512
                if last:
                    # Exposed tail: split the 1/Z scale onto the (now idle)
                    # scalar engine so it pipelines with the DVE bias adds.
                    nc.scalar.activation(
                        osb[:, 0:512], Tq[:, qo : qo + 512], AF.Copy,
                        scale=rz[:, t : t + 1],
                    )
                    nc.scalar.activation(
                        osb[:, 512:768], T2[:, t * 256 : (t + 1) * 256], AF.Copy,
                        scale=rz[:, t : t + 1],
                    )
                else:
                    nc.vector.tensor_scalar_mul(
                        osb[:, 0:512], Tq[:, qo : qo + 512], rz[:, t : t + 1]
                    )
                    nc.vector.tensor_scalar_mul(
                        osb[:, 512:768], T2[:, t * 256 : (t + 1) * 256],
                        rz[:, t : t + 1],
                    )
                nc.vector.tensor_add(osb[:, 0:512], osb[:, 0:512], boe_bc[:, 0:512])
                nc.vector.tensor_add(
                    osb[:, 512:768], osb[:, 512:768], boe_bc[:, 512:768]
                )
                ro = io + t * 128
                nc.sync.dma_start(out=out[ro : ro + 128, :], in_=osb)

        for p in (outpool, rzpool, zps, pvps, expool, stps, vpool, hpool,
                  wvopool, wapool, xpool, consts):
            p.release()

    nc.compile()
    _CACHE["nc"] = nc
    return nc


def _shard_inputs(x, wq, bq, wk, bk, wv, bv, wo, bo):
    """Build the 8 per-core input maps (host-side layout + weight algebra)."""
    f = np.float32
    f8 = np.float64
    h = np.float16
    x = np.asarray(x, f)
    wq, wk, wv, wo = (np.asarray(a, f8) for a in (wq, wk, wv, wo))
    bq, bk, bv, bo = (np.asarray(a, f8) for a in (bq, bk, bv, bo))

    def wtiles(w, dt):  # weight [out, in] -> [in-tile, 128, out]
        return np.ascontiguousarray(np.asarray(w, f).T).reshape(DT, 128, -1).astype(dt)

    A = (wq.T @ wk).astype(f)  # [d, e]; H = x @ A.T
    wvo = (wo @ wv).astype(f)  # fused value+out projection
    wkbq_col = (wk.T @ bq).astype(f)  # [768] -> w = x @ wkbq
    wvoT = wtiles(wvo, h)  # (DT, 128, D)
    wvoT_aug = np.concatenate(
        [wvoT, wkbq_col.reshape(DT, 128, 1).astype(h)], axis=2
    )  # (DT, 128, D+1)
    shared = {
        "waT": wtiles(A, h),
        "wvoT": np.ascontiguousarray(wvoT_aug),
        "boe": (bo + wo @ bv).astype(h).reshape(1, D),
    }
    in_maps = []
    for c in range(NCORES):
        b, half = c // 2, c % 2
        xb = np.ascontiguousarray(x[b].T)  # [D, S]
        if half:
            xb = np.concatenate([xb[:, QH:], xb[:, :QH]], axis=1)
        m = dict(shared)
        m["xT"] = np.ascontiguousarray(xb).reshape(DT, 128, S).astype(h)
        in_maps.append(m)
    return in_maps


def kernel(x, wq, bq, wk, bk, wv, bv, wo, bo, trace=False, trace_kwargs=None):
    global last_results
    from concourse.bass_utils import run_bass_kernel_spmd

    nc = _build_nc()
    in_maps = _shard_inputs(x, wq, bq, wk, bk, wv, bv, wo, bo)
    res = run_bass_kernel_spmd(
        nc,
        in_maps,
        core_ids=list(range(NCORES)),
        trace=trace,
        **(trace_kwargs or {}),
    )
    last_results = res
    out = np.empty((B, S, D), np.float32)
    for c in range(NCORES):
        b, half = c // 2, c % 2
        out[b, half * QH : (half + 1) * QH, :] = res.results[c]["out"].astype(np.float32)
    return out
